# revision 16
# baseline (speedup 1.0000x reference)
"""ContactMapHead Trainium2 kernel (v12: fp16, 256-col chunked stream,
fine-grained band schedule, PE preheat).

Reference computation (per batch b):
    h = relu(X @ W^T + pb)            # [S, DP]
    scores = (h @ h^T) * cw + cb      # [S, S]  -- symmetric!

Sharding over 8 NeuronCores: core c handles batch b = c//2 with roll
offset off = (c%2)*1024 applied to X on the host. Each core computes
hT = relu(WT^T @ XT + pb) for its full (rolled) batch, then emits the
circulant band of the symmetric score map: local tile rows i_t in 0..7
(tiles of 128), local cols j_t in i_t..i_t+8. Across the two cores of
a batch pair plus host-side transpose mirroring this covers all 16x16
global tiles exactly.

v12 vs v8a: X streams as 8 chunks of [P, KT, 256] (4KB lines, full DMA
rate) instead of 4 slabs of 512, so the first projection matmul starts
~2.5us earlier (ring + wt 1.45us + one 0.5MB chunk 1.45us). Trace
start-deltas prove N=256 fp16 matmuls sustain 1.01 cyc/row (the earlier
"LDWEIGHTS does not amortize below N=512" conclusion misread overlapped
durations). Band rows go out two-at-a-time after chunks 4-7, so the
output stream starts draining ~5us earlier and the tail shrinks.

Other measured constraints (see NTFF traces): power manager grants full
PE speed after ~5.2us sustained tensor activity (junk preheat covers
the DMA startup window); W must precede X on the same FIFO queue or it
starves; everything off-chip is fp16 with host-side pre-transpose so
the kernel has zero PE transposes.
"""

import numpy as np

from concourse import bacc, mybir, tile

P = 128
B, S, D = 4, 2048, 1024
DP = 256  # projection dim
NCORES = 8
KT = D // P  # 8 k-tiles over D
PT = DP // P  # 2 p-tiles over DP
CW = 256  # projection chunk width
NCH = S // CW  # 8 chunks
NROW = 8  # local band rows (tiles of 128) per core
BANDW = 9 * P  # 1152 band columns per row
SEG = BANDW // 3  # 384-col band chunks (fits one PSUM bank)
NPRE = 11  # preheat matmuls (512 rows each)

f32 = mybir.dt.float32
f16 = mybir.dt.float16


def _build_nc():
    nc = bacc.Bacc()
    x = nc.declare_dram_parameter("x", [NCH, P, KT, CW], f16, isOutput=False)
    w = nc.declare_dram_parameter("w", [P, KT, DP], f16, isOutput=False)
    pb = nc.declare_dram_parameter("pb", [DP], f32, isOutput=False)
    cwb = nc.declare_dram_parameter("cwb", [2], f32, isOutput=False)
    out = nc.declare_dram_parameter("out", [NROW, P, BANDW], f16, isOutput=True)

    with tile.TileContext(nc) as tc:
        _body(nc, tc, x, w, pb, cwb, out)
    nc.compile()
    return nc


def _body(nc, tc, x, w, pb, cwb, out):
    mult = mybir.AluOpType.mult
    add = mybir.AluOpType.add
    Relu = mybir.ActivationFunctionType.Relu
    Ident = mybir.ActivationFunctionType.Identity

    with (
        tc.tile_pool(name="const", bufs=1) as cpool,
        tc.tile_pool(name="orow", bufs=4) as opool,
        tc.tile_pool(name="ph", bufs=1, space="PSUM") as php,
        tc.tile_pool(name="pj", bufs=2, space="PSUM") as pj,
        tc.tile_pool(name="pw", bufs=3, space="PSUM") as pw,
    ):
        # ---- PE preheat: junk matmuls during the DMA startup window ----
        junk = cpool.tile([P, 512], f16, tag="junk")
        nc.gpsimd.memset(junk[:], 0.0)
        ph = php.tile([P, 512], f32, tag="ph")
        for _ in range(NPRE):
            nc.tensor.matmul(ph[:], junk[:, 0:P], junk[:], start=True, stop=True)

        # ---- small constants on the scalar queue ----
        pb_t = cpool.tile([P, PT], f32, tag="pb_t")
        nc.scalar.dma_start(pb_t[:], pb.ap().rearrange("(t p) -> p t", p=P))
        cwb_t = cpool.tile([P, 2], f32, tag="cwb_t")
        nc.scalar.dma_start(cwb_t[:], cwb.ap().partition_broadcast(P))

        # ---- W then X chunks on the sync queue in consumption order ----
        wt = cpool.tile([P, KT, DP], f16, tag="wt")
        nc.sync.dma_start(wt[:], w.ap())
        xall = cpool.tile([P, NCH, KT, CW], f16, tag="xall")
        for ch in range(NCH):
            nc.sync.dma_start(xall[:, ch], x.ap()[ch])

        # hT for the whole local map, fp16, dp on partitions
        ht = cpool.tile([P, PT, S], f16, tag="ht")

        def project(ch):
            o0 = ch * CW
            for pt in range(PT):
                pjs = pj.tile([P, CW], f32, tag="pj", name="pj")
                for k in range(KT):
                    nc.tensor.matmul(
                        pjs[:],
                        wt[:, k, pt * P : (pt + 1) * P],
                        xall[:, ch, k, :],
                        start=(k == 0),
                        stop=(k == KT - 1),
                    )
                if (ch + pt) % 2 == 0:
                    nc.scalar.activation(
                        ht[:, pt, o0 : o0 + CW],
                        pjs[:],
                        Relu,
                        bias=pb_t[:, pt : pt + 1],
                    )
                else:
                    nc.vector.tensor_scalar(
                        ht[:, pt, o0 : o0 + CW],
                        pjs[:],
                        pb_t[:, pt : pt + 1],
                        0.0,
                        add,
                        mybir.AluOpType.max,
                    )

        def emit_row(i_t):
            """Band row i_t: out[i_t] = cw * hT_i^T @ hT[band cols] + cb."""
            base = i_t * P
            psums = []
            for pt in range(PT):
                for si in range(3):
                    if pt == 0:
                        psums.append(pw.tile([P, SEG], f32, tag="pw", name="pw"))
                    c0 = base + si * SEG
                    nc.tensor.matmul(
                        psums[si][:],
                        ht[:, pt, base : base + P],
                        ht[:, pt, c0 : c0 + SEG],
                        start=(pt == 0),
                        stop=(pt == PT - 1),
                    )
            orow = opool.tile([P, BANDW], f16, tag="orow", name="orow")
            for si in range(3):
                dst = orow[:, si * SEG : (si + 1) * SEG]
                if (i_t * 3 + si) % 2 == 0:
                    nc.vector.tensor_scalar(
                        dst, psums[si][:], cwb_t[:, 0:1], cwb_t[:, 1:2], mult, add
                    )
                else:
                    nc.scalar.activation(
                        dst, psums[si][:], Ident,
                        bias=cwb_t[:, 1:2], scale=cwb_t[:, 0:1],
                    )
            eng = nc.sync if i_t % 2 == 0 else nc.scalar
            eng.dma_start(out.ap()[i_t], orow[:])

        # row i_t needs hT cols < i_t*128 + 1152; chunk ch provides cols
        # < (ch+1)*256  =>  rows 0,1 after ch4; 2,3 after ch5; 4,5 after
        # ch6; 6,7 after ch7.
        for ch in range(NCH):
            project(ch)
            if ch >= 4:
                emit_row((ch - 4) * 2)
                emit_row((ch - 4) * 2 + 1)


_NC_CACHE = None


def _get_nc():
    global _NC_CACHE
    if _NC_CACHE is None:
        _NC_CACHE = _build_nc()
    return _NC_CACHE


def _pack_xt(xb):
    """[S, D] float -> [NCH, P, KT, CW] fp16 with xd[ch,p,k,j] =
    X^T[k*128+p, ch*256+j]."""
    xt = xb.T.astype(np.float16)  # [D, S]
    return np.ascontiguousarray(
        xt.reshape(KT, P, NCH, CW).transpose(2, 1, 0, 3)
    )


def _make_in_maps(hidden_states, proj_w, proj_b, clf_w, clf_b):
    hs = np.asarray(hidden_states, dtype=np.float32)
    wv = np.asarray(proj_w, dtype=np.float32)
    pbv = np.ascontiguousarray(np.asarray(proj_b, dtype=np.float32).reshape(DP))
    cwbv = np.array(
        [np.asarray(clf_w).reshape(-1)[0], np.asarray(clf_b).reshape(-1)[0]],
        dtype=np.float32,
    )
    # wd[p, k, j] = W[j, k*128+p]
    wd = np.ascontiguousarray(
        wv.T.astype(np.float16).reshape(KT, P, DP).transpose(1, 0, 2)
    )
    in_maps = []
    for c in range(NCORES):
        b, half = divmod(c, 2)
        xb = hs[b]
        if half:
            xb = np.roll(xb, -S // 2, axis=0)
        in_maps.append({"x": _pack_xt(xb), "w": wd, "pb": pbv, "cwb": cwbv})
    return in_maps


def _assemble(results):
    scores = np.empty((B, S, S), np.float32)
    for c in range(NCORES):
        b, half = divmod(c, 2)
        o = results[c]["out"].astype(np.float32)  # [NROW, P, BANDW]
        for i_t in range(NROW):
            gi = i_t + NROW * half
            strip = o[i_t]
            for lj in range(i_t, i_t + 9):
                gj = (lj + NROW * half) % 16
                V = strip[:, (lj - i_t) * P : (lj - i_t + 1) * P]
                scores[b, gi * P : (gi + 1) * P, gj * P : (gj + 1) * P] = V
                if gj != gi:
                    scores[b, gj * P : (gj + 1) * P, gi * P : (gi + 1) * P] = V.T
    return scores


def kernel(hidden_states, proj_w, proj_b, clf_w, clf_b):
    from concourse.bass_utils import run_bass_kernel_spmd

    nc = _get_nc()
    in_maps = _make_in_maps(hidden_states, proj_w, proj_b, clf_w, clf_b)
    res = run_bass_kernel_spmd(nc, in_maps, core_ids=list(range(NCORES)))
    return _assemble(res.results)


def run_traced(hidden_states, proj_w, proj_b, clf_w, clf_b):
    """Like kernel(), but also returns BassKernelResults with trace info."""
    from concourse.bass_utils import run_bass_kernel_spmd

    nc = _get_nc()
    in_maps = _make_in_maps(hidden_states, proj_w, proj_b, clf_w, clf_b)
    res = run_bass_kernel_spmd(
        nc, in_maps, core_ids=list(range(NCORES)), trace=True
    )
    return _assemble(res.results), res


# revision 17
# speedup vs baseline: 1.0212x; 1.0212x over previous
"""ContactMapHead Trainium2 kernel (v12: fp16, 256-col chunked stream,
fine-grained band schedule, PE preheat).

Reference computation (per batch b):
    h = relu(X @ W^T + pb)            # [S, DP]
    scores = (h @ h^T) * cw + cb      # [S, S]  -- symmetric!

Sharding over 8 NeuronCores: core c handles batch b = c//2 with roll
offset off = (c%2)*1024 applied to X on the host. Each core computes
hT = relu(WT^T @ XT + pb) for its full (rolled) batch, then emits the
circulant band of the symmetric score map: local tile rows i_t in 0..7
(tiles of 128), local cols j_t in i_t..i_t+8. Across the two cores of
a batch pair plus host-side transpose mirroring this covers all 16x16
global tiles exactly.

v12 vs v8a: X streams as 8 chunks of [P, KT, 256] (4KB lines, full DMA
rate) instead of 4 slabs of 512, so the first projection matmul starts
~2.5us earlier (ring + wt 1.45us + one 0.5MB chunk 1.45us). Trace
start-deltas prove N=256 fp16 matmuls sustain 1.01 cyc/row (the earlier
"LDWEIGHTS does not amortize below N=512" conclusion misread overlapped
durations). Band rows go out two-at-a-time after chunks 4-7, so the
output stream starts draining ~5us earlier and the tail shrinks.

Other measured constraints (see NTFF traces): power manager grants full
PE speed after ~5.2us sustained tensor activity (junk preheat covers
the DMA startup window); W must precede X on the same FIFO queue or it
starves; everything off-chip is fp16 with host-side pre-transpose so
the kernel has zero PE transposes.
"""

import numpy as np

from concourse import bacc, mybir, tile

P = 128
B, S, D = 4, 2048, 1024
DP = 256  # projection dim
NCORES = 8
KT = D // P  # 8 k-tiles over D
PT = DP // P  # 2 p-tiles over DP
CW = 256  # projection chunk width
NCH = S // CW  # 8 chunks
NROW = 8  # local band rows (tiles of 128) per core
BANDW = 9 * P  # 1152 band columns per row
SEG = BANDW // 3  # 384-col band chunks (fits one PSUM bank)
NPRE = 56  # preheat matmuls (256 rows each: fine-grained so the
           # junk tail blocks the first real matmul by <~110ns and
           # the PE never idles into a power-state retraction)

f32 = mybir.dt.float32
f16 = mybir.dt.float16


def _build_nc():
    nc = bacc.Bacc()
    x = nc.declare_dram_parameter("x", [NCH, P, KT, CW], f16, isOutput=False)
    w = nc.declare_dram_parameter("w", [P, KT, DP], f16, isOutput=False)
    pb = nc.declare_dram_parameter("pb", [DP], f32, isOutput=False)
    cwb = nc.declare_dram_parameter("cwb", [2], f32, isOutput=False)
    out = nc.declare_dram_parameter("out", [NROW, P, BANDW], f16, isOutput=True)

    with tile.TileContext(nc) as tc:
        _body(nc, tc, x, w, pb, cwb, out)
    nc.compile()
    return nc


def _body(nc, tc, x, w, pb, cwb, out):
    mult = mybir.AluOpType.mult
    add = mybir.AluOpType.add
    Relu = mybir.ActivationFunctionType.Relu
    Ident = mybir.ActivationFunctionType.Identity

    with (
        tc.tile_pool(name="const", bufs=1) as cpool,
        tc.tile_pool(name="orow", bufs=4) as opool,
        tc.tile_pool(name="ph", bufs=1, space="PSUM") as php,
        tc.tile_pool(name="pj", bufs=2, space="PSUM") as pj,
        tc.tile_pool(name="pw", bufs=3, space="PSUM") as pw,
    ):
        # ---- PE preheat: junk matmuls during the DMA startup window ----
        junk = cpool.tile([P, 512], f16, tag="junk")
        nc.gpsimd.memset(junk[:], 0.0)
        ph = php.tile([P, 512], f32, tag="ph")
        for _ in range(NPRE):
            nc.tensor.matmul(
                ph[:, 0:256], junk[:, 0:P], junk[:, 0:256], start=True, stop=True
            )

        # ---- small constants on the scalar queue ----
        pb_t = cpool.tile([P, PT], f32, tag="pb_t")
        nc.scalar.dma_start(pb_t[:], pb.ap().rearrange("(t p) -> p t", p=P))
        cwb_t = cpool.tile([P, 2], f32, tag="cwb_t")
        nc.scalar.dma_start(cwb_t[:], cwb.ap().partition_broadcast(P))

        # ---- W then X chunks on the sync queue in consumption order ----
        wt = cpool.tile([P, KT, DP], f16, tag="wt")
        nc.sync.dma_start(wt[:], w.ap())
        xall = cpool.tile([P, NCH, KT, CW], f16, tag="xall")
        for ch in range(NCH):
            nc.sync.dma_start(xall[:, ch], x.ap()[ch])

        # hT for the whole local map, fp16, dp on partitions
        ht = cpool.tile([P, PT, S], f16, tag="ht")

        def project(ch):
            o0 = ch * CW
            for pt in range(PT):
                pjs = pj.tile([P, CW], f32, tag="pj", name="pj")
                for k in range(KT):
                    nc.tensor.matmul(
                        pjs[:],
                        wt[:, k, pt * P : (pt + 1) * P],
                        xall[:, ch, k, :],
                        start=(k == 0),
                        stop=(k == KT - 1),
                    )
                if (ch + pt) % 2 == 0:
                    nc.scalar.activation(
                        ht[:, pt, o0 : o0 + CW],
                        pjs[:],
                        Relu,
                        bias=pb_t[:, pt : pt + 1],
                    )
                else:
                    nc.vector.tensor_scalar(
                        ht[:, pt, o0 : o0 + CW],
                        pjs[:],
                        pb_t[:, pt : pt + 1],
                        0.0,
                        add,
                        mybir.AluOpType.max,
                    )

        def emit_row(i_t):
            """Band row i_t: out[i_t] = cw * hT_i^T @ hT[band cols] + cb."""
            base = i_t * P
            psums = []
            for pt in range(PT):
                for si in range(3):
                    if pt == 0:
                        psums.append(pw.tile([P, SEG], f32, tag="pw", name="pw"))
                    c0 = base + si * SEG
                    nc.tensor.matmul(
                        psums[si][:],
                        ht[:, pt, base : base + P],
                        ht[:, pt, c0 : c0 + SEG],
                        start=(pt == 0),
                        stop=(pt == PT - 1),
                    )
            orow = opool.tile([P, BANDW], f16, tag="orow", name="orow")
            for si in range(3):
                dst = orow[:, si * SEG : (si + 1) * SEG]
                if (i_t * 3 + si) % 2 == 0:
                    nc.vector.tensor_scalar(
                        dst, psums[si][:], cwb_t[:, 0:1], cwb_t[:, 1:2], mult, add
                    )
                else:
                    nc.scalar.activation(
                        dst, psums[si][:], Ident,
                        bias=cwb_t[:, 1:2], scale=cwb_t[:, 0:1],
                    )
            eng = nc.sync if i_t % 2 == 0 else nc.scalar
            eng.dma_start(out.ap()[i_t], orow[:])

        # row i_t needs hT cols < i_t*128 + 1152; chunk ch provides cols
        # < (ch+1)*256  =>  rows 0,1 after ch4; 2,3 after ch5; 4,5 after
        # ch6; 6,7 after ch7.
        for ch in range(NCH):
            project(ch)
            if ch >= 4:
                emit_row((ch - 4) * 2)
                emit_row((ch - 4) * 2 + 1)


_NC_CACHE = None


def _get_nc():
    global _NC_CACHE
    if _NC_CACHE is None:
        _NC_CACHE = _build_nc()
    return _NC_CACHE


def _pack_xt(xb):
    """[S, D] float -> [NCH, P, KT, CW] fp16 with xd[ch,p,k,j] =
    X^T[k*128+p, ch*256+j]."""
    xt = xb.T.astype(np.float16)  # [D, S]
    return np.ascontiguousarray(
        xt.reshape(KT, P, NCH, CW).transpose(2, 1, 0, 3)
    )


def _make_in_maps(hidden_states, proj_w, proj_b, clf_w, clf_b):
    hs = np.asarray(hidden_states, dtype=np.float32)
    wv = np.asarray(proj_w, dtype=np.float32)
    pbv = np.ascontiguousarray(np.asarray(proj_b, dtype=np.float32).reshape(DP))
    cwbv = np.array(
        [np.asarray(clf_w).reshape(-1)[0], np.asarray(clf_b).reshape(-1)[0]],
        dtype=np.float32,
    )
    # wd[p, k, j] = W[j, k*128+p]
    wd = np.ascontiguousarray(
        wv.T.astype(np.float16).reshape(KT, P, DP).transpose(1, 0, 2)
    )
    in_maps = []
    for c in range(NCORES):
        b, half = divmod(c, 2)
        xb = hs[b]
        if half:
            xb = np.roll(xb, -S // 2, axis=0)
        in_maps.append({"x": _pack_xt(xb), "w": wd, "pb": pbv, "cwb": cwbv})
    return in_maps


def _assemble(results):
    scores = np.empty((B, S, S), np.float32)
    for c in range(NCORES):
        b, half = divmod(c, 2)
        o = results[c]["out"].astype(np.float32)  # [NROW, P, BANDW]
        for i_t in range(NROW):
            gi = i_t + NROW * half
            strip = o[i_t]
            for lj in range(i_t, i_t + 9):
                gj = (lj + NROW * half) % 16
                V = strip[:, (lj - i_t) * P : (lj - i_t + 1) * P]
                scores[b, gi * P : (gi + 1) * P, gj * P : (gj + 1) * P] = V
                if gj != gi:
                    scores[b, gj * P : (gj + 1) * P, gi * P : (gi + 1) * P] = V.T
    return scores


def kernel(hidden_states, proj_w, proj_b, clf_w, clf_b):
    from concourse.bass_utils import run_bass_kernel_spmd

    nc = _get_nc()
    in_maps = _make_in_maps(hidden_states, proj_w, proj_b, clf_w, clf_b)
    res = run_bass_kernel_spmd(nc, in_maps, core_ids=list(range(NCORES)))
    return _assemble(res.results)


def run_traced(hidden_states, proj_w, proj_b, clf_w, clf_b):
    """Like kernel(), but also returns BassKernelResults with trace info."""
    from concourse.bass_utils import run_bass_kernel_spmd

    nc = _get_nc()
    in_maps = _make_in_maps(hidden_states, proj_w, proj_b, clf_w, clf_b)
    res = run_bass_kernel_spmd(
        nc, in_maps, core_ids=list(range(NCORES)), trace=True
    )
    return _assemble(res.results), res


# revision 18
# speedup vs baseline: 1.0303x; 1.0089x over previous
"""ContactMapHead Trainium2 kernel (v8: fp16, host-pretransposed, slab-
streamed DMA in consumption order, PE preheat, early output drain).

Reference computation (per batch b):
    h = relu(X @ W^T + pb)            # [S, DP]
    scores = (h @ h^T) * cw + cb      # [S, S]  -- symmetric!

Sharding over 8 NeuronCores: core c handles batch b = c//2 with roll
offset off = (c%2)*1024 applied to X on the host. Each core computes
hT = relu(WT^T @ XT + pb) for its full (rolled) batch, then emits the
circulant band of the symmetric score map: local tile rows i_t in 0..7
(tiles of 128), local cols j_t in i_t..i_t+8 (9 tiles of 128, never
wrapping). Across the two cores of a batch pair plus host-side
transpose mirroring this covers all 16x16 global tiles exactly.

Pipeline design (v8):
  - All X/W/out DRAM traffic is fp16 (half the bytes of fp32; fp16
    matmul is 1 cyc/row). X and W are pre-transposed/pre-tiled on the
    host so the kernel does zero PE transposes and every DMA line is
    2-8KB contiguous per partition.
  - W streams first, then X as 4 slabs of [P, KT, 512], all on the
    sync queue in exact consumption order; outputs alternate queues.
  - Projection uses N=512 matmuls: at N=256 the per-matmul LDWEIGHTS
    does not amortize (measured 2.5 cyc/row effective); at N=512 it
    overlaps (1.06 cyc/row measured).
  - Band rows 0-1 are emitted before the sb3 projection so the output
    DMA (2.25 MiB) starts draining ~10us before the PE finishes.
  - Junk matmuls keep the PE busy from the earliest dispatch point:
    the power manager grants full PE speed only after ~5.2us of
    sustained tensor activity (retracts after ~2us idle), so the ramp
    must overlap the DMA startup window and never gap afterwards.
"""

import numpy as np

from concourse import bacc, mybir, tile

P = 128
B, S, D = 4, 2048, 1024
DP = 256  # projection dim
NCORES = 8
KT = D // P  # 8 k-tiles over D
PT = DP // P  # 2 p-tiles over DP
SBLK = 512
NSB = S // SBLK  # 4 s-blocks
NROW = 8  # local band rows (tiles of 128) per core
BANDW = 9 * P  # 1152 band columns per row
SEG = BANDW // 3  # 384-col band chunks (fits one PSUM bank)
NPRE = 15  # preheat matmuls (512 rows each)

f32 = mybir.dt.float32
f16 = mybir.dt.float16


def _build_nc():
    nc = bacc.Bacc()
    x = nc.declare_dram_parameter("x", [NSB, P, KT, SBLK], f16, isOutput=False)
    w = nc.declare_dram_parameter("w", [P, KT, DP], f16, isOutput=False)
    pb = nc.declare_dram_parameter("pb", [DP], f32, isOutput=False)
    cwb = nc.declare_dram_parameter("cwb", [2], f32, isOutput=False)
    out = nc.declare_dram_parameter("out", [NROW, P, BANDW], f16, isOutput=True)

    with tile.TileContext(nc) as tc:
        _body(nc, tc, x, w, pb, cwb, out)
    nc.compile()
    return nc


def _body(nc, tc, x, w, pb, cwb, out):
    mult = mybir.AluOpType.mult
    add = mybir.AluOpType.add
    Relu = mybir.ActivationFunctionType.Relu
    Ident = mybir.ActivationFunctionType.Identity

    with (
        tc.tile_pool(name="const", bufs=1) as cpool,
        tc.tile_pool(name="orow", bufs=4) as opool,
        tc.tile_pool(name="ph", bufs=1, space="PSUM") as php,
        tc.tile_pool(name="pj", bufs=2, space="PSUM") as pj,
        tc.tile_pool(name="pw", bufs=3, space="PSUM") as pw,
    ):
        # ---- PE preheat: junk matmuls during the DMA startup window ----
        junk = cpool.tile([P, SBLK], f16, tag="junk")
        nc.gpsimd.memset(junk[:], 0.0)
        ph = php.tile([P, SBLK], f32, tag="ph")
        for _ in range(NPRE):
            nc.tensor.matmul(ph[:], junk[:, 0:P], junk[:], start=True, stop=True)

        # ---- small constants on the scalar queue ----
        pb_t = cpool.tile([P, PT], f32, tag="pb_t")
        nc.scalar.dma_start(pb_t[:], pb.ap().rearrange("(t p) -> p t", p=P))
        cwb_t = cpool.tile([P, 2], f32, tag="cwb_t")
        nc.scalar.dma_start(cwb_t[:], cwb.ap().partition_broadcast(P))

        # ---- W then X slabs on the sync queue in consumption order ----
        wt = cpool.tile([P, KT, DP], f16, tag="wt")
        nc.sync.dma_start(wt[:], w.ap())
        xall = cpool.tile([P, NSB, KT, SBLK], f16, tag="xall")
        for sb in range(NSB):
            nc.sync.dma_start(xall[:, sb], x.ap()[sb])

        # hT for the whole local map, fp16, dp on partitions
        ht = cpool.tile([P, PT, S], f16, tag="ht")

        def project(sb):
            o0 = sb * SBLK
            for pt in range(PT):
                pjs = pj.tile([P, SBLK], f32, tag="pj", name="pj")
                for k in range(KT):
                    nc.tensor.matmul(
                        pjs[:],
                        wt[:, k, pt * P : (pt + 1) * P],
                        xall[:, sb, k, :],
                        start=(k == 0),
                        stop=(k == KT - 1),
                    )
                if (sb + pt) % 2 == 0:
                    nc.scalar.activation(
                        ht[:, pt, o0 : o0 + SBLK],
                        pjs[:],
                        Relu,
                        bias=pb_t[:, pt : pt + 1],
                    )
                else:
                    nc.vector.tensor_scalar(
                        ht[:, pt, o0 : o0 + SBLK],
                        pjs[:],
                        pb_t[:, pt : pt + 1],
                        0.0,
                        add,
                        mybir.AluOpType.max,
                    )

        def emit_row(i_t):
            """Band row i_t: out[i_t] = cw * hT_i^T @ hT[band cols] + cb."""
            base = i_t * P
            psums = []
            for pt in range(PT):
                for si in range(3):
                    if pt == 0:
                        psums.append(pw.tile([P, SEG], f32, tag="pw", name="pw"))
                    c0 = base + si * SEG
                    nc.tensor.matmul(
                        psums[si][:],
                        ht[:, pt, base : base + P],
                        ht[:, pt, c0 : c0 + SEG],
                        start=(pt == 0),
                        stop=(pt == PT - 1),
                    )
            orow = opool.tile([P, BANDW], f16, tag="orow", name="orow")
            for si in range(3):
                dst = orow[:, si * SEG : (si + 1) * SEG]
                if (i_t * 3 + si) % 2 == 0:
                    nc.vector.tensor_scalar(
                        dst, psums[si][:], cwb_t[:, 0:1], cwb_t[:, 1:2], mult, add
                    )
                else:
                    nc.scalar.activation(
                        dst, psums[si][:], Ident,
                        bias=cwb_t[:, 1:2], scale=cwb_t[:, 0:1],
                    )
            eng = nc.sync if i_t % 2 == 0 else nc.scalar
            eng.dma_start(out.ap()[i_t], orow[:])

        # rows 0-3 need hT cols < 1536 (sb0-2); rows 4-7 need sb3 too.
        # Emit rows 0-1 before the sb3 projection so output DMA starts
        # draining as early as possible.
        project(0)
        project(1)
        project(2)
        emit_row(0)
        emit_row(1)
        project(3)
        emit_row(2)
        emit_row(3)
        for i_t in range(4, NROW):
            emit_row(i_t)


_NC_CACHE = None


def _get_nc():
    global _NC_CACHE
    if _NC_CACHE is None:
        _NC_CACHE = _build_nc()
    return _NC_CACHE


def _pack_xt(xb):
    """[S, D] float -> [NSB, P, KT, SBLK] fp16 with xd[sb,p,k,j] =
    X^T[k*128+p, sb*512+j]."""
    xt = xb.T.astype(np.float16)  # [D, S]
    return np.ascontiguousarray(
        xt.reshape(KT, P, NSB, SBLK).transpose(2, 1, 0, 3)
    )


def _make_in_maps(hidden_states, proj_w, proj_b, clf_w, clf_b):
    hs = np.asarray(hidden_states, dtype=np.float32)
    wv = np.asarray(proj_w, dtype=np.float32)
    pbv = np.ascontiguousarray(np.asarray(proj_b, dtype=np.float32).reshape(DP))
    cwbv = np.array(
        [np.asarray(clf_w).reshape(-1)[0], np.asarray(clf_b).reshape(-1)[0]],
        dtype=np.float32,
    )
    # wd[p, k, j] = W[j, k*128+p]
    wd = np.ascontiguousarray(
        wv.T.astype(np.float16).reshape(KT, P, DP).transpose(1, 0, 2)
    )
    in_maps = []
    for c in range(NCORES):
        b, half = divmod(c, 2)
        xb = hs[b]
        if half:
            xb = np.roll(xb, -S // 2, axis=0)
        in_maps.append({"x": _pack_xt(xb), "w": wd, "pb": pbv, "cwb": cwbv})
    return in_maps


def _assemble(results):
    scores = np.empty((B, S, S), np.float32)
    for c in range(NCORES):
        b, half = divmod(c, 2)
        o = results[c]["out"].astype(np.float32)  # [NROW, P, BANDW]
        for i_t in range(NROW):
            gi = i_t + NROW * half
            strip = o[i_t]
            for lj in range(i_t, i_t + 9):
                gj = (lj + NROW * half) % 16
                V = strip[:, (lj - i_t) * P : (lj - i_t + 1) * P]
                scores[b, gi * P : (gi + 1) * P, gj * P : (gj + 1) * P] = V
                if gj != gi:
                    scores[b, gj * P : (gj + 1) * P, gi * P : (gi + 1) * P] = V.T
    return scores


def kernel(hidden_states, proj_w, proj_b, clf_w, clf_b):
    from concourse.bass_utils import run_bass_kernel_spmd

    nc = _get_nc()
    in_maps = _make_in_maps(hidden_states, proj_w, proj_b, clf_w, clf_b)
    res = run_bass_kernel_spmd(nc, in_maps, core_ids=list(range(NCORES)))
    return _assemble(res.results)


def run_traced(hidden_states, proj_w, proj_b, clf_w, clf_b):
    """Like kernel(), but also returns BassKernelResults with trace info."""
    from concourse.bass_utils import run_bass_kernel_spmd

    nc = _get_nc()
    in_maps = _make_in_maps(hidden_states, proj_w, proj_b, clf_w, clf_b)
    res = run_bass_kernel_spmd(
        nc, in_maps, core_ids=list(range(NCORES)), trace=True
    )
    return _assemble(res.results), res


# revision 21
# speedup vs baseline: 1.0743x; 1.0427x over previous
"""ContactMapHead Trainium2 kernel (v13: fp16, host-pretransposed, slab-
streamed DMA in consumption order, PE preheat, early output drain).

Reference computation (per batch b):
    h = relu(X @ W^T + pb)            # [S, DP]
    scores = (h @ h^T) * cw + cb      # [S, S]  -- symmetric!

Sharding over 8 NeuronCores: core c handles batch b = c//2 with roll
offset off = (c%2)*1024 applied to X on the host. Each core computes
hT = relu(WT^T @ XT + pb) for its full (rolled) batch, then emits the
circulant band of the symmetric score map: local tile rows i_t in 0..7
(tiles of 128), local cols j_t in i_t..i_t+8 (9 tiles of 128, never
wrapping). Across the two cores of a batch pair plus host-side
transpose mirroring this covers all 16x16 global tiles exactly.

Pipeline design (v8):
  - All X/W/out DRAM traffic is fp16 (half the bytes of fp32; fp16
    matmul is 1 cyc/row). X and W are pre-transposed/pre-tiled on the
    host so the kernel does zero PE transposes and every DMA line is
    2-8KB contiguous per partition.
  - W streams first, then X as 4 slabs of [P, KT, 512], all on the
    sync queue in exact consumption order; outputs alternate queues.
  - Projection uses N=512 matmuls: at N=256 the per-matmul LDWEIGHTS
    does not amortize (measured 2.5 cyc/row effective); at N=512 it
    overlaps (1.06 cyc/row measured).
  - Band rows 0-1 are emitted before the sb3 projection so the output
    DMA (2.25 MiB) starts draining ~10us before the PE finishes.
  - Junk matmuls keep the PE busy from the earliest dispatch point:
    the power manager grants full PE speed only after ~5.2us of
    sustained tensor activity (retracts after ~2us idle), so the ramp
    must overlap the DMA startup window and never gap afterwards.
"""

import numpy as np

from concourse import bacc, mybir, tile

P = 128
B, S, D = 4, 2048, 1024
DP = 256  # projection dim
NCORES = 8
KT = D // P  # 8 k-tiles over D
PT = DP // P  # 2 p-tiles over DP
SBLK = 512
NSB = S // SBLK  # 4 s-blocks
NROW = 8  # local band rows (tiles of 128) per core
BANDW = 9 * P  # 1152 band columns per row
SEG = BANDW // 3  # 384-col band chunks (fits one PSUM bank)
NPRE = 15  # preheat matmuls (512 rows each)

f32 = mybir.dt.float32
f16 = mybir.dt.float16


def _build_nc():
    nc = bacc.Bacc()
    x = nc.declare_dram_parameter("x", [NSB, P, KT, SBLK], f16, isOutput=False)
    w = nc.declare_dram_parameter("w", [PT, P, KT, P], f16, isOutput=False)
    pb = nc.declare_dram_parameter("pb", [DP], f32, isOutput=False)
    cwb = nc.declare_dram_parameter("cwb", [2], f32, isOutput=False)
    out = nc.declare_dram_parameter("out", [NROW, P, BANDW], f16, isOutput=True)

    with tile.TileContext(nc) as tc:
        _body(nc, tc, x, w, pb, cwb, out)
    nc.compile()
    return nc


def _body(nc, tc, x, w, pb, cwb, out):
    mult = mybir.AluOpType.mult
    add = mybir.AluOpType.add
    Relu = mybir.ActivationFunctionType.Relu
    Ident = mybir.ActivationFunctionType.Identity

    with (
        tc.tile_pool(name="const", bufs=1) as cpool,
        tc.tile_pool(name="orow", bufs=5) as opool,
        tc.tile_pool(name="ph", bufs=1, space="PSUM") as php,
        tc.tile_pool(name="pj", bufs=2, space="PSUM") as pj,
        tc.tile_pool(name="pw", bufs=4, space="PSUM") as pw,
    ):
        # ---- PE preheat: junk matmuls during the DMA startup window ----
        junk = cpool.tile([P, SBLK], f16, tag="junk")
        nc.gpsimd.memset(junk[:], 0.0)
        ph = php.tile([P, SBLK], f32, tag="ph")
        for _ in range(NPRE):
            nc.tensor.matmul(ph[:], junk[:, 0:P], junk[:], start=True, stop=True)

        # ---- small constants on the scalar queue ----
        pb_t = cpool.tile([P, PT], f32, tag="pb_t")
        nc.scalar.dma_start(pb_t[:], pb.ap().rearrange("(t p) -> p t", p=P))
        cwb_t = cpool.tile([P, 2], f32, tag="cwb_t")
        nc.scalar.dma_start(cwb_t[:], cwb.ap().partition_broadcast(P))

        # ---- W then X slabs on the sync queue in consumption order ----
        wts = [
            cpool.tile([P, KT, P], f16, tag=f"wt{pt}", name=f"wt{pt}")
            for pt in range(PT)
        ]
        xall = cpool.tile([P, NSB, KT, SBLK], f16, tag="xall")
        # wt0 gates the very first matmul; wt1 is only needed ~1.7us
        # later, so it rides between slab0 and slab1.
        nc.sync.dma_start(wts[0][:], w.ap()[0])
        nc.sync.dma_start(xall[:, 0], x.ap()[0])
        nc.sync.dma_start(wts[1][:], w.ap()[1])
        for sb in range(1, NSB):
            nc.sync.dma_start(xall[:, sb], x.ap()[sb])

        # hT for the whole local map, fp16, dp on partitions
        ht = cpool.tile([P, PT, S], f16, tag="ht")

        def project(sb):
            o0 = sb * SBLK
            for pt in range(PT):
                pjs = pj.tile([P, SBLK], f32, tag="pj", name="pj")
                for k in range(KT):
                    nc.tensor.matmul(
                        pjs[:],
                        wts[pt][:, k, :],
                        xall[:, sb, k, :],
                        start=(k == 0),
                        stop=(k == KT - 1),
                    )
                if (sb + pt) % 2 == 0:
                    nc.scalar.activation(
                        ht[:, pt, o0 : o0 + SBLK],
                        pjs[:],
                        Relu,
                        bias=pb_t[:, pt : pt + 1],
                    )
                else:
                    nc.vector.tensor_scalar(
                        ht[:, pt, o0 : o0 + SBLK],
                        pjs[:],
                        pb_t[:, pt : pt + 1],
                        0.0,
                        add,
                        mybir.AluOpType.max,
                    )

        def emit_row(i_t):
            """Band row i_t: out[i_t] = cw * hT_i^T @ hT[band cols] + cb."""
            base = i_t * P
            psums = []
            for pt in range(PT):
                for si in range(3):
                    if pt == 0:
                        psums.append(pw.tile([P, SEG], f32, tag="pw", name="pw"))
                    c0 = base + si * SEG
                    nc.tensor.matmul(
                        psums[si][:],
                        ht[:, pt, base : base + P],
                        ht[:, pt, c0 : c0 + SEG],
                        start=(pt == 0),
                        stop=(pt == PT - 1),
                    )
            orow = opool.tile([P, BANDW], f16, tag="orow", name="orow")
            for si in range(3):
                dst = orow[:, si * SEG : (si + 1) * SEG]
                if (i_t * 3 + si) % 2 == 0:
                    nc.vector.tensor_scalar(
                        dst, psums[si][:], cwb_t[:, 0:1], cwb_t[:, 1:2], mult, add
                    )
                else:
                    nc.scalar.activation(
                        dst, psums[si][:], Ident,
                        bias=cwb_t[:, 1:2], scale=cwb_t[:, 0:1],
                    )
            eng = nc.sync if i_t % 2 == 0 else nc.scalar
            eng.dma_start(out.ap()[i_t], orow[:])

        # rows 0-3 need hT cols < 1536 (sb0-2); rows 4-7 need sb3 too.
        # Emit rows 0-1 before the sb3 projection so output DMA starts
        # draining as early as possible.
        project(0)
        project(1)
        project(2)
        emit_row(0)
        emit_row(1)
        project(3)
        emit_row(2)
        emit_row(3)
        for i_t in range(4, NROW):
            emit_row(i_t)


_NC_CACHE = None


def _get_nc():
    global _NC_CACHE
    if _NC_CACHE is None:
        _NC_CACHE = _build_nc()
    return _NC_CACHE


def _pack_xt(xb):
    """[S, D] float -> [NSB, P, KT, SBLK] fp16 with xd[sb,p,k,j] =
    X^T[k*128+p, sb*512+j]."""
    xt = xb.T.astype(np.float16)  # [D, S]
    return np.ascontiguousarray(
        xt.reshape(KT, P, NSB, SBLK).transpose(2, 1, 0, 3)
    )


def _make_in_maps(hidden_states, proj_w, proj_b, clf_w, clf_b):
    hs = np.asarray(hidden_states, dtype=np.float32)
    wv = np.asarray(proj_w, dtype=np.float32)
    pbv = np.ascontiguousarray(np.asarray(proj_b, dtype=np.float32).reshape(DP))
    cwbv = np.array(
        [np.asarray(clf_w).reshape(-1)[0], np.asarray(clf_b).reshape(-1)[0]],
        dtype=np.float32,
    )
    # wd[pt, p, k, j] = W[pt*128+j, k*128+p]
    wd = np.ascontiguousarray(
        wv.T.astype(np.float16)
        .reshape(KT, P, PT, P)
        .transpose(2, 1, 0, 3)
    )
    in_maps = []
    for c in range(NCORES):
        b, half = divmod(c, 2)
        xb = hs[b]
        if half:
            xb = np.roll(xb, -S // 2, axis=0)
        in_maps.append({"x": _pack_xt(xb), "w": wd, "pb": pbv, "cwb": cwbv})
    return in_maps


def _assemble(results):
    scores = np.empty((B, S, S), np.float32)
    for c in range(NCORES):
        b, half = divmod(c, 2)
        o = results[c]["out"].astype(np.float32)  # [NROW, P, BANDW]
        for i_t in range(NROW):
            gi = i_t + NROW * half
            strip = o[i_t]
            for lj in range(i_t, i_t + 9):
                gj = (lj + NROW * half) % 16
                V = strip[:, (lj - i_t) * P : (lj - i_t + 1) * P]
                scores[b, gi * P : (gi + 1) * P, gj * P : (gj + 1) * P] = V
                if gj != gi:
                    scores[b, gj * P : (gj + 1) * P, gi * P : (gi + 1) * P] = V.T
    return scores


def kernel(hidden_states, proj_w, proj_b, clf_w, clf_b):
    from concourse.bass_utils import run_bass_kernel_spmd

    nc = _get_nc()
    in_maps = _make_in_maps(hidden_states, proj_w, proj_b, clf_w, clf_b)
    res = run_bass_kernel_spmd(nc, in_maps, core_ids=list(range(NCORES)))
    return _assemble(res.results)


def run_traced(hidden_states, proj_w, proj_b, clf_w, clf_b):
    """Like kernel(), but also returns BassKernelResults with trace info."""
    from concourse.bass_utils import run_bass_kernel_spmd

    nc = _get_nc()
    in_maps = _make_in_maps(hidden_states, proj_w, proj_b, clf_w, clf_b)
    res = run_bass_kernel_spmd(
        nc, in_maps, core_ids=list(range(NCORES)), trace=True
    )
    return _assemble(res.results), res
